# revision 11
# baseline (speedup 1.0000x reference)
"""Autoformer encoder (nn_Autoformer_11441792876586) on 8 TRN2 NeuronCores.

Strategy: data-parallel over batch (4 batches/core). Per core, everything runs
in feature-major layout (channels on partitions, time on free dim):
  - all GEMMs: W stationary (din,dout)-tiles, activations moving -> fp16 in,
    fp32 PSUM accumulate
  - AutoCorrelation mean_corr via Gram matrix M = Q^T K (c-contraction),
    block-diagonal sums C_j, DRAM shear read (diagonal access pattern) and a
    ones-vector matmul partition reduction; top-8 via DVE Max8/MaxIndex;
    softmax on ACT/DVE; time-rolls of V@Wo via dynamic-start APs (register
    offsets) into a doubled buffer
  - series_decomp via DVE prefix-scan cumsum + edge-replication corrections
  - final my_Layernorm folded: out = (z - mean_t z) @ (g*proj_w) + proj_b
"""
import sys
if "/opt/trn_rl_repo" not in sys.path:
    sys.path.insert(0, "/opt/trn_rl_repo")
import numpy as np

L, CIN, D, DFF = 1024, 64, 512, 2048
NL_TOT, KMA, TOPK, EPS = 3, 25, 6, 1e-5
NCORES = 8
PAD = (KMA - 1) // 2  # 12
NTC = 2               # 512-wide time chunks per 1024
_BUILD_CACHE = {}


STAGES = ["embed", "qkv", "gram", "cj", "shear", "svec", "r", "topk",
          "combine", "dec1", "ffn", "full"]


def _build(nbatch, nlayers, stop_stage="full"):
    _stop = STAGES.index(stop_stage)
    def _ge(st):
        return STAGES.index(st) <= _stop
    import concourse.bass as bass
    from concourse.bass import _add_dep_helper
    import concourse.bacc as bacc
    import concourse.tile as tile
    import concourse.mybir as mybir
    from contextlib import ExitStack

    DT = mybir.dt
    AF = mybir.ActivationFunctionType
    ALU = mybir.AluOpType
    AX = mybir.AxisListType
    F16, F32 = DT.float16, DT.float32

    nc = bacc.Bacc("TRN2", target_bir_lowering=False, debug=False,
                   num_devices=NCORES)

    # ---------------- I/O ----------------
    x_e = nc.dram_tensor("x", [nbatch, 64, L + 2], F16, kind="ExternalInput")
    wqkvo_e = nc.dram_tensor("wqkvo", [nlayers, 4, 4, 128, 512], F16, kind="ExternalInput")
    w1_e = nc.dram_tensor("w1", [nlayers, 4, 128, DFF], F16, kind="ExternalInput")
    w2_e = nc.dram_tensor("w2", [nlayers, 16, 128, 512], F16, kind="ExternalInput")
    ew_e = nc.dram_tensor("ew", [3, 64, 512], F16, kind="ExternalInput")
    bias_e = nc.dram_tensor("biases", [128, nlayers * 16], F32, kind="ExternalInput")
    projw_e = nc.dram_tensor("projw", [4, 128, 64], F16, kind="ExternalInput")
    pbt_e = nc.dram_tensor("pbt", [128, 64], F32, kind="ExternalInput")
    rampf_e = nc.dram_tensor("rampf", [128, PAD], F32, kind="ExternalInput")
    rampb_e = nc.dram_tensor("rampb", [128, PAD], F32, kind="ExternalInput")
    ones_e = nc.dram_tensor("ones512", [128, 1], F16, kind="ExternalInput")
    basef_e = nc.dram_tensor("basef", [128, 64], F32, kind="ExternalInput")
    out_e = nc.dram_tensor("out", [nbatch, L, 64], F32, kind="ExternalOutput")
    # internal DRAM shear buffers (alternating parity per batch)
    dsh = [nc.dram_tensor(f"dsh{p}", [8, 128, 512], F16) for p in range(2)]

    with tile.TileContext(nc) as tc, ExitStack() as ctx:
        pool = ctx.enter_context(tc.tile_pool(name="sb", bufs=1))
        wpool = ctx.enter_context(tc.tile_pool(name="wp", bufs=1))
        cspool = ctx.enter_context(tc.tile_pool(name="cs", bufs=1))
        bigpool = ctx.enter_context(tc.tile_pool(name="big", bufs=1))
        pg = ctx.enter_context(tc.tile_pool(name="pg", bufs=4, space="PSUM"))
        pm = ctx.enter_context(tc.tile_pool(name="pm", bufs=2, space="PSUM"))

        # ------------- persistent constants -------------
        bias_sb = pool.tile([128, nlayers * 16], F32, tag="bias")
        nc.sync.dma_start(bias_sb[:], bias_e.ap())
        ew_sb = pool.tile([64, 3 * 512], F16, tag="ew")
        nc.sync.dma_start(ew_sb[:].rearrange("p (j c) -> p j c", j=3),
                          ew_e.ap().rearrange("j p c -> p j c"))
        projw_sb = pool.tile([128, 4 * 64], F16, tag="projw")
        nc.sync.dma_start(projw_sb[:].rearrange("p (k c) -> p k c", k=4),
                          projw_e.ap().rearrange("k p c -> p k c"))
        pbt_sb = pool.tile([128, 64], F32, tag="pbt")
        nc.sync.dma_start(pbt_sb[:], pbt_e.ap())
        rampf_sb = pool.tile([128, PAD], F32, tag="rampf")
        nc.sync.dma_start(rampf_sb[:], rampf_e.ap())
        rampb_sb = pool.tile([128, PAD], F32, tag="rampb")
        nc.sync.dma_start(rampb_sb[:], rampb_e.ap())
        ones_sb = pool.tile([128, 1], F16, tag="ones")
        nc.sync.dma_start(ones_sb[:], ones_e.ap())
        basef_sb = pool.tile([128, 64], F32, tag="basef")
        nc.sync.dma_start(basef_sb[:], basef_e.ap())

        # zero the shear scratch in DRAM once
        zero_sb = pool.tile([128, 512], F16, tag="xin")
        nc.gpsimd.memset(zero_sb[:], 0.0)
        zero_writes = {0: [], 1: []}
        shear_writes = [{}, {}]
        shear_reads = [{}, {}]
        for p in range(2):
            for j in range(8):
                zero_writes[p].append(nc.sync.dma_start(dsh[p].ap()[j], zero_sb[:]))

        # persistent per-batch h (fp32, feature-major: c-tile m at cols [1024m))
        h_b = [pool.tile([128, 4 * L], F32, tag=f"h{b}", name=f"h{b}")
               for b in range(nbatch)]

        def cast_h16(b):
            h16 = pool.tile([128, 4 * L], F16, tag="h16")
            for m in range(4):
                nc.vector.tensor_copy(h16[:, m * L:(m + 1) * L],
                                      h_b[b][:, m * L:(m + 1) * L])
            return h16

        def gemm_512(dst_sb, dst_col, w_sb, w_base, rhs_sb, bias_ap=None,
                     act=None, nk=4, dst_dup=None):
            """dst[:, dst_col + m*L + tc*512] = act(sum_k W[k,m]^T @ rhs[k,tc]) + bias
            W blocks at w_sb[:, w_base + 512k + 128m]; rhs c-tile k at rhs_sb cols
            [L*k], time chunk tc at [512tc]. dst layout: c-tile m at [L*m]."""
            for m in range(4):
                for t in range(NTC):
                    ps = pg.tile([128, 512], F32, tag="pg")
                    for k in range(nk):
                        nc.tensor.matmul(
                            ps[:],
                            w_sb[:, w_base + 512 * k + 128 * m:
                                 w_base + 512 * k + 128 * m + 128],
                            rhs_sb[:, L * k + 512 * t: L * k + 512 * t + 512],
                            start=(k == 0), stop=(k == nk - 1))
                    col = dst_col + L * m + 512 * t
                    nc.scalar.activation(dst_sb[:, col:col + 512], ps[:],
                                         act or AF.Identity,
                                         bias=bias_ap[m] if bias_ap else 0.0)
                    if dst_dup is not None:
                        nc.vector.tensor_copy(
                            dst_sb[:, col + dst_dup:col + dst_dup + 512],
                            dst_sb[:, col:col + 512])

        def bias_aps(l, w):
            return [bias_sb[:, l * 16 + w * 4 + m: l * 16 + w * 4 + m + 1]
                    for m in range(4)]

        def decomp(b):
            """h <- (h) - moving_average(h) with edge replication; h=(128,4L) f32."""
            h = h_b[b]
            for m in range(4):
                hx = h[:, m * L:(m + 1) * L]
                cs = pool.tile([128, L + 2 * PAD + 4], F32, tag="s_sb", name="cs")
                nc.gpsimd.memset(cs[:, 0:PAD + 1], 0.0)
                nc.vector.tensor_tensor_scan(cs[:, PAD + 1:PAD + 1 + L], hx, hx,
                                             0.0, ALU.add, ALU.bypass)
                # replicate cs[L] into the back pad
                nc.scalar.activation(
                    cs[:, PAD + 1 + L:PAD + 1 + L + PAD],
                    cs[:, PAD + L:PAD + 1 + L].to_broadcast((128, PAD)),
                    AF.Identity)
                # save edge values of hx before overwrite
                ecol = cspool.tile([128, 2], F32, tag="ecol")
                nc.vector.tensor_copy(ecol[:, 0:1], hx[:, 0:1])
                nc.vector.tensor_copy(ecol[:, 1:2], hx[:, L - 1:L])
                # windowed sum A[t] = cs[t+25] - cs[t] (padded cs)
                tmp = pool.tile([128, L], F32, tag="t16", name="tmp")
                nc.vector.tensor_tensor(tmp[:], cs[:, KMA:KMA + L], cs[:, 0:L],
                                        ALU.subtract)
                # h = hx - A/25
                nc.vector.scalar_tensor_tensor(hx, tmp[:], -1.0 / KMA, hx,
                                               ALU.mult, ALU.add)
                # edge corrections (ramps are pre-negated/25 on host)
                nc.vector.scalar_tensor_tensor(
                    hx[:, 0:PAD], rampf_sb[:], ecol[:, 0:1], hx[:, 0:PAD],
                    ALU.mult, ALU.add)
                nc.vector.scalar_tensor_tensor(
                    hx[:, L - PAD:L], rampb_sb[:], ecol[:, 1:2], hx[:, L - PAD:L],
                    ALU.mult, ALU.add)

        # tensor_tensor helper name check: bass engines expose tensor_tensor?
        # (nc.vector.tensor_tensor(out, in0, in1, op)) -- see bass.py

        # ================= embed =================
        for b in range(nbatch):
            x_sb = pool.tile([64, L + 2], F16, tag="xin")
            nc.sync.dma_start(x_sb[:], x_e.ap()[b])
            for m in range(4):
                for t in range(NTC):
                    ps = pg.tile([128, 512], F32, tag="pg")
                    for j in range(3):
                        nc.tensor.matmul(
                            ps[:],
                            ew_sb[0:64, 512 * j + 128 * m: 512 * j + 128 * m + 128],
                            x_sb[0:64, j + 512 * t: j + 512 * t + 512],
                            start=(j == 0), stop=(j == 2))
                    nc.scalar.activation(
                        h_b[b][:, L * m + 512 * t: L * m + 512 * t + 512],
                        ps[:], AF.Identity)

        # ================= layers =================
        for l in range(nlayers):
            qkvo_sb = wpool.tile([128, 4 * 2048], F16, tag="qkvo")
            nc.sync.dma_start(
                qkvo_sb[:].rearrange("p (w k c) -> p w k c", w=4, k=4),
                wqkvo_e.ap()[l].rearrange("w k p c -> p w k c"))
            ffn_sb = wpool.tile([128, 16384], F16, tag="ffn")
            nc.sync.dma_start(
                ffn_sb[:, 0:8192].rearrange("p (k c) -> p k c", k=4),
                w1_e.ap()[l].rearrange("k p c -> p k c"))
            nc.sync.dma_start(
                ffn_sb[:, 8192:16384].rearrange("p (k c) -> p k c", k=16),
                w2_e.ap()[l].rearrange("k p c -> p k c"))

            for b in range(nbatch):
                h16 = cast_h16(b)
                if not _ge("qkv"):
                    continue
                q16 = pool.tile([128, 4 * L], F16, tag="q16")
                k16 = pool.tile([128, 4 * L], F16, tag="k16")
                v16 = pool.tile([128, 4 * L], F16, tag="v16")
                vo32 = pool.tile([128, 4 * L], F32, tag="vo2")
                gemm_512(q16, 0, qkvo_sb, 0, h16, bias_aps(l, 0))
                gemm_512(k16, 0, qkvo_sb, 2048, h16, bias_aps(l, 1))
                gemm_512(v16, 0, qkvo_sb, 4096, h16, bias_aps(l, 2))
                # vo = v @ wo (+bo), fp32 (rolled later via ap_gather)
                gemm_512(vo32, 0, qkvo_sb, 6144, v16, bias_aps(l, 3))

                if not _ge("gram"):
                    continue
                # ---- Gram M = Q^T K (fp16), tiles i at M16 cols [1024 i) ----
                m16 = bigpool.tile([128, 8 * L], F16, tag="big")
                for i in range(8):
                    psm = pm.tile([128, 1024], F32, tag="pm")
                    for t2 in range(2):
                        for kc in range(4):
                            nc.tensor.matmul(
                                psm[:, 512 * t2:512 * t2 + 512],
                                q16[:, L * kc + 128 * i: L * kc + 128 * i + 128],
                                k16[:, L * kc + 512 * t2: L * kc + 512 * t2 + 512],
                                start=(kc == 0), stop=(kc == 3))
                    nc.vector.tensor_copy(m16[:, L * i:L * i + 1024], psm[:])

                if not _ge("cj"):
                    continue
                # ---- C_j ----
                cacc = pool.tile([128, 1024], F32, tag="cacc")
                cw16 = pool.tile([128, 1024], F16, tag="cw16")
                for j in range(8):
                    dst32 = cacc[:, 128 * j:128 * j + 128]
                    for i in range(8):
                        blk = m16[:, L * i + 128 * ((i - j) % 8):
                                  L * i + 128 * ((i - j) % 8) + 128]
                        if i == 0:
                            nc.vector.tensor_copy(dst32, blk)
                        elif i < 7:
                            nc.vector.tensor_tensor(dst32, dst32, blk, ALU.add)
                        else:
                            nc.vector.tensor_tensor(
                                cw16[:, 128 * j:128 * j + 128], dst32, blk,
                                ALU.add)
                    wr = nc.sync.dma_start(dsh[b % 2].ap()[j, :, 256:384],
                                           cw16[:, 128 * j:128 * j + 128])
                    for prd in shear_reads[b % 2].get(j, []):
                        _add_dep_helper(wr.ins, prd.ins, sync=True,
                                        reason="shear WAR")
                    shear_writes[b % 2][j] = wr

                if not _ge("shear"):
                    continue
                # ---- shear read T[p, (j,u)] = D[j, p, p - u + 383] ----
                t16 = pool.tile([128, 8 * 256], F16, tag="t16")
                for j in range(8):
                    shear_in = bass.AP(
                        tensor=dsh[b % 2].ap().tensor,
                        offset=383 + 128 * 512 * j,
                        ap=[[513, 128], [-1, 256]])
                    rd = nc.sync.dma_start(t16[:, 256 * j:256 * j + 256], shear_in)
                    _add_dep_helper(rd.ins, shear_writes[b % 2][j].ins,
                                    sync=True, reason="shear RAW")
                    for zw in zero_writes[b % 2]:
                        _add_dep_helper(rd.ins, zw.ins, sync=True,
                                        reason="shear zero RAW")
                    shear_reads[b % 2].setdefault(j, []).append(rd)

                if not _ge("svec"):
                    continue
                # ---- s_j = ones(1/512)^T @ T_j  -> SBUF row ----
                s_sb = pool.tile([1, 8 * 256], F32, tag="s_sb")
                for j in range(8):
                    pss = pg.tile([1, 256], F32, tag="pg")
                    nc.tensor.matmul(pss[:], ones_sb[:],
                                     t16[:, 256 * j:256 * j + 256],
                                     start=True, stop=True)
                    nc.vector.tensor_copy(s_sb[:, 256 * j:256 * j + 256], pss[:])

                if not _ge("r"):
                    continue
                # ---- assemble r[128j+t] = s_j[127+t] + s_{j+1}[t-1] ----
                r32 = pool.tile([1, L], F32, tag="r32")
                src_a = bass.AP(tensor=s_sb[:].tensor, offset=s_sb[:].offset + 127,
                                ap=[[s_sb[:].ap[0][0], 1], [256, 8], [1, 128]])
                nc.vector.tensor_copy(
                    r32[:].rearrange("p (j t) -> p j t", j=8), src_a)
                # += s_{j+1}[t-1] for t>=1 ; j=0..6
                dst_b = bass.AP(tensor=r32[:].tensor, offset=r32[:].offset + 1,
                                ap=[[r32[:].ap[0][0], 1], [128, 7], [1, 127]])
                src_b = bass.AP(tensor=s_sb[:].tensor, offset=s_sb[:].offset + 256,
                                ap=[[s_sb[:].ap[0][0], 1], [256, 7], [1, 127]])
                nc.vector.tensor_tensor(dst_b, dst_b, src_b, ALU.add)
                # j=7 wraps to s_0
                nc.vector.tensor_tensor(r32[:, 897:1024], r32[:, 897:1024],
                                        s_sb[:, 0:127], ALU.add)

                if not _ge("topk"):
                    continue
                # ---- top-8 + softmax over top-6 ----
                top8 = pool.tile([1, 8], F32, tag="top8")
                idx8 = pool.tile([1, 8], DT.uint32, tag="idx8")
                nc.vector.max(top8[:], r32[:])
                nc.vector.max_index(idx8[:], top8[:], r32[:])
                negmax = pool.tile([1, 1], F32, tag="negmax")
                nc.vector.tensor_scalar_mul(negmax[:], top8[:, 0:1], -1.0)
                e6 = pool.tile([1, 8], F32, tag="e6")
                nc.scalar.activation(e6[:, 0:TOPK], top8[:, 0:TOPK], AF.Exp,
                                     bias=negmax[:], scale=1.0)
                se = pool.tile([1, 1], F32, tag="se")
                nc.vector.tensor_reduce(se[:], e6[:, 0:TOPK], AX.X, ALU.add)
                rse = pool.tile([1, 1], F32, tag="rse")
                nc.vector.reciprocal(rse[:], se[:])
                tc6 = pool.tile([1, 8], F32, tag="tc6")
                nc.vector.tensor_scalar_mul(tc6[:, 0:TOPK], e6[:, 0:TOPK], rse[:])
                tcb = pool.tile([128, 8], F32, tag="tcb")
                nc.gpsimd.partition_broadcast(tcb[:, 0:TOPK], tc6[:, 0:TOPK],
                                              channels=128)

                if not _ge("combine"):
                    continue
                # ---- combine ----
                for k in range(TOPK):
                    db = pool.tile([128, 1], DT.uint32, tag="db", name="db")
                    nc.gpsimd.partition_broadcast(db[:], idx8[0:1, k:k + 1],
                                                  channels=128)
                    d32 = pool.tile([128, 1], F32, tag="d32", name="d32")
                    nc.vector.tensor_copy(d32[:], db[:].bitcast(DT.int32))
                    idxf = pool.tile([128, 64], F32, tag="idxf", name="idxf")
                    nc.vector.tensor_scalar(idxf[:], basef_sb[:], d32[:], None,
                                            ALU.add)
                    mask = pool.tile([128, 64], F32, tag="mask", name="mask")
                    nc.vector.tensor_scalar(mask[:], idxf[:], float(L), None,
                                            ALU.is_ge)
                    nc.vector.scalar_tensor_tensor(idxf[:], mask[:], -float(L),
                                                   idxf[:], ALU.mult, ALU.add)
                    idxs = pool.tile([128, 64], DT.int16, tag="idxs", name="idxs")
                    nc.vector.tensor_copy(idxs[:], idxf[:])
                    for m in range(4):
                        rolled = pool.tile([128, L], F32, tag="t16",
                                           name="rolled")
                        nc.gpsimd.ap_gather(rolled[:],
                                            vo32[:, L * m:L * m + L], idxs[:],
                                            channels=128, num_elems=L, d=1,
                                            num_idxs=L)
                        nc.vector.scalar_tensor_tensor(
                            h_b[b][:, L * m:L * m + L], rolled[:],
                            tcb[:, k:k + 1], h_b[b][:, L * m:L * m + L],
                            ALU.mult, ALU.add)

                if not _ge("dec1"):
                    continue
                decomp(b)

                if not _ge("ffn"):
                    continue
                # ================= FFN =================
                h16f = cast_h16(b)
                for t in range(NTC):
                    g16 = bigpool.tile([128, 16 * 512], F16, tag="big")
                    for dm in range(16):
                        ps = pg.tile([128, 512], F32, tag="pg")
                        for k in range(4):
                            nc.tensor.matmul(
                                ps[:],
                                ffn_sb[:, 2048 * k + 128 * dm:
                                       2048 * k + 128 * dm + 128],
                                h16f[:, L * k + 512 * t: L * k + 512 * t + 512],
                                start=(k == 0), stop=(k == 3))
                        nc.scalar.activation(g16[:, 512 * dm:512 * dm + 512],
                                             ps[:], AF.Gelu)
                    for m in range(4):
                        psy = pg.tile([128, 512], F32, tag="pg")
                        for k in range(16):
                            nc.tensor.matmul(
                                psy[:],
                                ffn_sb[:, 8192 + 512 * k + 128 * m:
                                       8192 + 512 * k + 128 * m + 128],
                                g16[:, 512 * k:512 * k + 512],
                                start=(k == 0), stop=(k == 15))
                        col = L * m + 512 * t
                        nc.vector.tensor_tensor(h_b[b][:, col:col + 512],
                                                h_b[b][:, col:col + 512],
                                                psy[:], ALU.add)
                decomp(b)

        # ================= final LN + projection =================
        for b in range(nbatch if _ge("full") else 0):
            h16 = cast_h16(b)
            # mu, sumsq rows via ones-matmuls (scaled by 1/512)
            stat = pool.tile([1, 2 * L], F32, tag="vo2")  # [mu | ssq]
            h2 = bigpool.tile([128, 4 * L], F16, tag="big")
            for m in range(4):
                nc.scalar.activation(h2[:, L * m:L * m + L],
                                     h16[:, L * m:L * m + L], AF.Square)
            for which, src in ((0, h16), (1, h2)):
                for t in range(NTC):
                    ps = pg.tile([1, 512], F32, tag="pg")
                    for k in range(4):
                        nc.tensor.matmul(ps[:], ones_sb[:],
                                         src[:, L * k + 512 * t: L * k + 512 * t + 512],
                                         start=(k == 0), stop=(k == 3))
                    nc.vector.tensor_copy(
                        stat[:, which * L + 512 * t: which * L + 512 * t + 512],
                        ps[:])
            # var = ssq - mu^2 ; inv = 1/sqrt(var+eps)
            mu2 = pool.tile([1, L], F32, tag="r32")
            nc.vector.tensor_tensor(mu2[:], stat[:, 0:L], stat[:, 0:L], ALU.mult)
            var = pool.tile([1, L], F32, tag="t16")
            nc.vector.tensor_tensor(var[:], stat[:, L:2 * L], mu2[:], ALU.subtract)
            nc.vector.tensor_scalar_add(var[:], var[:], EPS)
            sd = pool.tile([1, L], F32, tag="cacc")
            nc.scalar.activation(sd[:], var[:], AF.Sqrt)
            inv = pool.tile([1, L], F32, tag="s_sb")
            nc.vector.reciprocal(inv[:], sd[:])
            # broadcast mu, inv to 128 partitions
            mub = pool.tile([128, L], F32, tag="q16")
            nc.gpsimd.partition_broadcast(mub[:], stat[:, 0:L], channels=128)
            invb = pool.tile([128, L], F32, tag="v16")
            nc.gpsimd.partition_broadcast(invb[:], inv[:], channels=128)
            # z = (h - mu) * inv ; z -= mean_t(z)
            z16 = pool.tile([128, 4 * L], F16, tag="k16")
            for m in range(4):
                zf = pool.tile([128, L], F32, tag="t16", name="zf")
                nc.vector.tensor_tensor(zf[:], h_b[b][:, L * m:L * m + L], mub[:],
                                        ALU.subtract)
                nc.vector.tensor_tensor(zf[:], zf[:], invb[:], ALU.mult)
                mz = pool.tile([128, 1], F32, tag="mz")
                nc.vector.tensor_reduce(mz[:], zf[:], AX.X, ALU.add)
                nc.vector.tensor_scalar_mul(mz[:], mz[:], 1.0 / L)
                nc.vector.tensor_scalar_sub(z16[:, L * m:L * m + L], zf[:], mz[:])
            # out[t, co] = z^T @ projw (z tiles stationary) + pb
            o32 = pool.tile([128, 8 * 64], F32, tag="cw16")
            for tt in range(8):
                ps = pg.tile([128, 64], F32, tag="pg")
                for k in range(4):
                    nc.tensor.matmul(ps[:],
                                     z16[:, L * k + 128 * tt: L * k + 128 * tt + 128],
                                     projw_sb[:, 64 * k:64 * k + 64],
                                     start=(k == 0), stop=(k == 3))
                nc.vector.tensor_tensor(o32[:, 64 * tt:64 * tt + 64], ps[:],
                                        pbt_sb[:], ALU.add)
                nc.sync.dma_start(out_e.ap()[b, 128 * tt:128 * tt + 128, :],
                                  o32[:, 64 * tt:64 * tt + 64])

        if not _ge("full"):
            # dump h (f32) of batch 0 into out[0..] rows for inspection
            dump = pool.tile([128, 1024], F32, tag="cw16", name="dump")
            nc.vector.tensor_copy(dump[:], h_b[0][:, 0:1024])
            nc.sync.dma_start(out_e.ap()[0, 0:128, 0:64], dump[:, 0:64])

    nc.compile()
    return nc


def _get_program(nbatch=4, nlayers=NL_TOT, stop_stage="full"):
    key = (nbatch, nlayers, stop_stage)
    if key not in _BUILD_CACHE:
        _BUILD_CACHE[key] = _build(nbatch, nlayers, stop_stage)
    return _BUILD_CACHE[key]


def _prep_shared(inputs, nlayers):
    """Host-side input marshalling shared by all cores (weight layout/cast)."""
    f16 = np.float16
    wqkvo = np.stack([np.stack([np.asarray(inputs[n][l]).reshape(4, 128, 512)
                                for n in ("wq", "wk", "wv", "wo")])
                      for l in range(nlayers)]).astype(f16)
    w1 = np.stack([np.asarray(inputs["w1"][l]).reshape(4, 128, DFF)
                   for l in range(nlayers)]).astype(f16)
    w2 = np.stack([np.asarray(inputs["w2"][l]).reshape(16, 128, 512)
                   for l in range(nlayers)]).astype(f16)
    ew = np.asarray(inputs["embed_w"]).astype(f16)
    biases = np.zeros((128, nlayers * 16), np.float32)
    for l in range(nlayers):
        for w, n in enumerate(("bq", "bk", "bv", "bo")):
            arr = np.asarray(inputs[n][l])
            for m in range(4):
                biases[:, l * 16 + w * 4 + m] = arr[m * 128:(m + 1) * 128]
    projw = (np.asarray(inputs["ln_g"])[:, None]
             * np.asarray(inputs["proj_w"])).reshape(4, 128, 64).astype(f16)
    pbt = np.tile(np.asarray(inputs["proj_b"])[None, :], (128, 1)).astype(np.float32)
    rampf = np.tile((-(PAD - np.arange(PAD)) / KMA)[None, :], (128, 1)).astype(np.float32)
    rampb = np.tile((-(np.arange(PAD) + 1) / KMA)[None, :], (128, 1)).astype(np.float32)
    ones512 = np.full((128, 1), 1.0 / 512, f16)
    basef = ((np.arange(64)[None, :] * 16)
             + (np.arange(128)[:, None] % 16)).astype(np.float32)
    return dict(wqkvo=wqkvo, w1=w1, w2=w2, ew=ew, biases=biases, projw=projw,
                pbt=pbt, rampf=rampf, rampb=rampb, ones512=ones512, basef=basef)


def _prep_x(xb):
    """(nb, L, CIN) fp32 -> (nb, 64, L+2) fp16 feature-major, circular padded."""
    xt = np.transpose(np.asarray(xb), (0, 2, 1))  # (nb, C, L)
    xe = np.concatenate([xt[:, :, -1:], xt, xt[:, :, :1]], axis=2)
    return xe.astype(np.float16)


def kernel(**inputs):
    from concourse.bass_utils import run_bass_kernel_spmd
    x = np.asarray(inputs["x"])
    B = x.shape[0]
    nbatch = B // NCORES
    nc = _get_program(nbatch, NL_TOT)
    shared = _prep_shared(inputs, NL_TOT)
    in_maps = []
    for c in range(NCORES):
        m = dict(shared)
        m["x"] = _prep_x(x[c * nbatch:(c + 1) * nbatch])
        in_maps.append(m)
    res = run_bass_kernel_spmd(nc, in_maps, core_ids=list(range(NCORES)))
    out = np.concatenate([res.results[c]["out"] for c in range(NCORES)], axis=0)
    return out.astype(np.float32)
